# revision 7
# baseline (speedup 1.0000x reference)
"""Trainium2 Bass kernel for nn_MultiHeadAttention (B=2, Nq=Nk=2048, D=1024, H=16).

Sharding: head-parallel across 8 cores (2 heads/core, both batches) for the
projections + attention, then an AllToAll redistributes the tiny per-head
context vectors so the epilogue (Wo + bias + residual + LayerNorm) runs
token-parallel (256 tokens/batch/core).

Key design points:
 - scores are computed transposed (s^T[k,q] = kT.T @ qT, contraction d=64
   with the core's two heads row-packed into disjoint PE row groups), so the
   huge attention_weights tensor is loaded in a host-pre-transposed layout
   and P^T feeds the P@V matmul directly as the PE moving operand.
 - softmax denominators come from an extra ones-column in the stationary
   [V | 1] operand (no separate reduction pass).
 - masking: the host builds wz = w*(1-mask)/8 with exact zeros where masked;
   exp then yields exactly 1 there, and an accumulated matmul of [-V | -1]
   against the 0/1 mask (recomputed on-device as wz==0 by the otherwise-idle
   GPSIMD) removes those contributions exactly. No max-subtraction is needed
   (|q.k * w / 8| is O(6), far from exp overflow).
 - two sequential TileContexts (attention / epilogue) so SBUF pools are
   reused across the phases.
"""
import sys

sys.path.insert(0, "/opt/trn_rl_repo")

import contextlib

import numpy as np

import concourse.bass as bass
import concourse.mybir as mybir
import concourse.tile as tile
from concourse.vector_clock import ScopedClock

# ---------------------------------------------------------------------------
# Workaround: this walrus build accepts only ONE sync-wait on CTRL (Drain)
# instructions; the Tile tail drain can carry several. Split the surplus
# waits across trailing sync NOPs (they still run before the sem reset).
# ---------------------------------------------------------------------------
_MAX_WAITS = 1


def _split_waits(inst, nc):
    si = inst.sync_info
    if si is None or not si.on_wait or len(si.on_wait) <= _MAX_WAITS:
        return
    waits = list(si.on_wait)
    inst.sync_info = mybir.SyncInfo(
        on_wait=waits[:_MAX_WAITS], on_update=list(si.on_update)
    )
    for i in range(_MAX_WAITS, len(waits), _MAX_WAITS):
        nop = nc.sync.nop(nofuse=True)
        nsi = nop.ins.sync_info
        upd = list(nsi.on_update) if nsi is not None else []
        nop.ins.sync_info = mybir.SyncInfo(
            on_wait=waits[i : i + _MAX_WAITS], on_update=upd
        )


def _patched_drain_and_barrier(self, tick_clock, wait_clock):
    nc = self.nc
    drain_inst = nc.sync.drain()
    wait_clock.add_sem_waits(
        drain_inst.ins, ScopedClock({None: tick_clock.global_clock})
    )
    _split_waits(drain_inst.ins, nc)
    nc.all_engine_barrier()
    assert self.sems is not None
    popped = nc._tile_sem_poison_stack.pop()
    assert popped is self._sem_poison
    nc.clear_and_free_semaphores(list(self.sems.allocated().values()))
    nc.all_engine_barrier()


tile.TileContext._drain_and_barrier = _patched_drain_and_barrier


def _fixup_sync_waits(nc, max_waits=1):
    """Split any instruction carrying more than `max_waits` semaphore waits
    into same-engine NOPs placed immediately before it (this walrus build
    rejects multi-wait instructions)."""
    n_split = 0
    for f in nc.m.functions:
        for blk in f.blocks:
            insts = blk.instructions
            new = []
            for ins in insts:
                si = getattr(ins, "sync_info", None)
                if si is not None and si.on_wait and len(si.on_wait) > max_waits:
                    waits = list(si.on_wait)
                    head = waits[:-max_waits]
                    for k in range(0, len(head), max_waits):
                        nop = mybir.InstNoOp(
                            name=f"{ins.name}.wsplit{k}",
                            engine=ins.engine,
                            bass_nofuse=True,
                            sync_info=mybir.SyncInfo(
                                on_wait=head[k : k + max_waits], on_update=[]
                            ),
                        )
                        new.append(nop)
                        n_split += 1
                    ins.sync_info = mybir.SyncInfo(
                        on_wait=waits[-max_waits:], on_update=list(si.on_update)
                    )
                new.append(ins)
            if len(new) != len(insts):
                insts[:] = new
    return n_split

# ---------------------------------------------------------------------------

F32 = mybir.dt.float32
AF = mybir.ActivationFunctionType
OP = mybir.AluOpType

B, H, NQ, NK, D, DH = 2, 16, 2048, 2048, 1024, 64
NC = 8
HPC = H // NC          # heads per core = 2
TPC = NQ // NC         # epilogue tokens per core per batch = 256
LN_EPS = 1e-5

KT = NK // 128         # 16 k-tiles
QC = NQ // 512         # 4 q-chunks
DT = D // 128          # 8 dmodel tiles


def _build_bass():
    nc = bass.Bass(num_devices=NC)

    # ---- parameters -------------------------------------------------------
    xtq = nc.declare_dram_parameter("xtq", [B, D, NQ], F32, isOutput=False)
    xtk = nc.declare_dram_parameter("xtk", [B, D, NK], F32, isOutput=False)
    xtv = nc.declare_dram_parameter("xtv", [B, D, NK], F32, isOutput=False)
    wzt = nc.declare_dram_parameter("wzt", [B, HPC, NK, NQ], F32, isOutput=False)
    wqt = nc.declare_dram_parameter("wqt", [D, 128], F32, isOutput=False)
    wkt = nc.declare_dram_parameter("wkt", [D, 128], F32, isOutput=False)
    wvt = nc.declare_dram_parameter("wvt", [D, 128], F32, isOutput=False)
    wot = nc.declare_dram_parameter("wot", [D, D], F32, isOutput=False)
    bq = nc.declare_dram_parameter("bq", [128, 1], F32, isOutput=False)
    bk = nc.declare_dram_parameter("bk", [128, 1], F32, isOutput=False)
    bv = nc.declare_dram_parameter("bv", [1, 128], F32, isOutput=False)
    bo = nc.declare_dram_parameter("bo", [1, D], F32, isOutput=False)
    gamma = nc.declare_dram_parameter("gamma", [1, D], F32, isOutput=False)
    beta = nc.declare_dram_parameter("beta", [1, D], F32, isOutput=False)
    qres = nc.declare_dram_parameter("qres", [B, TPC, D], F32, isOutput=False)
    out_c = nc.declare_dram_parameter("out_c", [B, TPC, D], F32, isOutput=True)

    # DRAM scratch shared across the two tile contexts.
    # a2a_in[g, b, :, :] = this core's heads, token chunk destined to core g.
    # After AllToAll: a2a_out[g, b, :, :] = core g's heads for THIS core's
    # token slice.
    a2a_in = nc.dram_tensor("a2a_in", [NC, B, 128, TPC], F32)
    a2a_out = nc.dram_tensor("a2a_out", [NC, B, 128, TPC], F32)

    # =======================================================================
    # Phase 1: projections + attention (head-parallel)
    # =======================================================================
    with tile.TileContext(nc) as tc, contextlib.ExitStack() as ctx:
        consts = ctx.enter_context(tc.tile_pool(name="consts", bufs=1))
        sb = ctx.enter_context(tc.tile_pool(name="sb", bufs=1))
        psum = ctx.enter_context(tc.tile_pool(name="psum", bufs=1, space="PSUM"))

        ones_row = consts.tile([1, 128], F32)
        nc.vector.memset(ones_row[:], 1.0)
        ones_col64 = consts.tile([1, 64], F32)
        nc.vector.memset(ones_col64[:], 1.0)

        wq_sb = consts.tile([128, DT, 128], F32)
        nc.sync.dma_start(wq_sb[:], wqt.rearrange("(o p) m -> p o m", p=128))
        wk_sb = consts.tile([128, DT, 128], F32)
        nc.sync.dma_start(wk_sb[:], wkt.rearrange("(o p) m -> p o m", p=128))
        wv_sb = consts.tile([128, DT, 128], F32)
        nc.sync.dma_start(wv_sb[:], wvt.rearrange("(o p) m -> p o m", p=128))

        bq_sb = consts.tile([128, 1], F32)
        nc.sync.dma_start(bq_sb[:], bq[:])
        bk_sb = consts.tile([128, 1], F32)
        nc.sync.dma_start(bk_sb[:], bk[:])
        bv_sb = consts.tile([1, 128], F32)
        nc.sync.dma_start(bv_sb[:], bv[:])

        for b in range(B):
            # ---- Q/K projections, transposed layout [hd, tok] ------------
            qt_sb = sb.tile([128, NQ], F32, tag="qt_sb", name="qt_sb")
            kt_sb = sb.tile([128, NK], F32, tag="kt_sb", name="kt_sb")
            for src, dst, w_sb, b_ap in (
                (xtq, qt_sb, wq_sb, bq_sb),
                (xtk, kt_sb, wk_sb, bk_sb),
            ):
                for half in range(2):
                    hslc = slice(half * 1024, (half + 1) * 1024)
                    pj = [
                        psum.tile([128, 512], F32, tag="pp", bufs=2, name="pj")
                        for _ in range(2)
                    ]
                    for kd in range(DT):
                        xs = sb.tile([128, 1024], F32, tag="xs", bufs=3,
                                     name="xs")
                        nc.sync.dma_start(
                            xs[:], src[b, kd * 128 : (kd + 1) * 128, hslc]
                        )
                        for i in range(2):
                            nc.tensor.matmul(
                                pj[i][:], w_sb[:, kd, :],
                                xs[:, i * 512 : (i + 1) * 512],
                                start=(kd == 0), stop=(kd == DT - 1),
                            )
                    for i in range(2):
                        tsl = slice(half * 1024 + i * 512,
                                    half * 1024 + (i + 1) * 512)
                        nc.scalar.activation(
                            dst[:, tsl], pj[i][:], AF.Identity,
                            bias=b_ap[:], scale=1.0,
                        )

            # ---- V projection, natural layout [tok, hd]; build [V|1] and
            # [-V|-1] per head for the PV + mask-correction matmuls.
            vexts = []
            vnegs = []
            for hl in range(HPC):
                ve = sb.tile([128, KT, 65], F32, tag=f"vext{hl}", name="ve")
                nc.vector.memset(ve[:, :, 64:65], 1.0)
                vn = sb.tile([128, KT, 65], F32, tag=f"vneg{hl}", name="vn")
                nc.vector.memset(vn[:, :, 64:65], -1.0)
                vexts.append(ve)
                vnegs.append(vn)
            for tt in range(KT):
                xv8 = sb.tile([128, DT, 128], F32, tag="xv8", bufs=2,
                              name="xv8")
                nc.sync.dma_start(
                    xv8[:],
                    xtv[b].rearrange("(o p) t -> p o t", p=128)[
                        :, :, tt * 128 : (tt + 1) * 128
                    ],
                )
                pv = psum.tile([128, 128], F32, tag="pp", bufs=2, name="pv")
                for kd in range(DT):
                    nc.tensor.matmul(
                        pv[:], xv8[:, kd, :], wv_sb[:, kd, :],
                        start=(kd == 0), stop=False,
                    )
                nc.tensor.matmul(
                    pv[:], ones_row[:, 0:128], bv_sb[:],
                    start=False, stop=True,
                )
                for hl in range(HPC):
                    nc.scalar.copy(
                        vexts[hl][:, tt, 0:64], pv[:, hl * 64 : hl * 64 + 64]
                    )
                    nc.vector.tensor_scalar_mul(
                        vnegs[hl][:, tt, 0:64],
                        pv[:, hl * 64 : hl * 64 + 64], -1.0,
                    )

            # ---- attention ----------------------------------------------
            outTn_sb = sb.tile([128, NQ], F32, tag="outTn", name="outTn_sb")
            for qc in range(QC):
                qsl = slice(qc * 512, (qc + 1) * 512)
                oacc = [
                    psum.tile([65, 512], F32, tag=f"o{hl}", name="oacc")
                    for hl in range(HPC)
                ]
                for g in range(KT // 2):
                    texp = [
                        sb.tile([128, 1024], F32, tag=f"t{hl}", bufs=2,
                                name="texp")
                        for hl in range(HPC)
                    ]
                    mts = {}
                    for j in range(2):
                        kt = 2 * g + j
                        ksl = slice(kt * 128, (kt + 1) * 128)
                        jsl = slice(j * 512, (j + 1) * 512)
                        for hl in range(HPC):
                            hsl = slice(hl * 64, (hl + 1) * 64)
                            wz = sb.tile([128, 512], F32, tag=f"wz{hl}",
                                         bufs=4, name="wz")
                            nc.sync.dma_start(wz[:], wzt[b, hl, ksl, qsl])
                            mt = sb.tile([128, 512], F32, tag=f"mt{hl}",
                                         bufs=4, name="mt")
                            nc.gpsimd.tensor_scalar(
                                mt[:], wz[:], 0.0, None, OP.is_equal
                            )
                            mts[(j, hl)] = mt
                            ps = psum.tile([128, 512], F32, tag=f"ps{hl}",
                                           bufs=2, name="ps")
                            nc.tensor.matmul(
                                ps[:], kt_sb[hsl, ksl], qt_sb[hsl, qsl],
                                start=True, stop=True,
                                tile_position=(hl * 64, 0),
                            )
                            nc.vector.tensor_tensor(
                                texp[hl][:, jsl], ps[:], wz[:], OP.mult
                            )
                    for hl in range(HPC):
                        pt = sb.tile([128, 1024], F32, tag=f"pt{hl}", bufs=2,
                                     name="pt")
                        nc.scalar.activation(pt[:], texp[hl][:], AF.Exp)
                        for j in range(2):
                            kt = 2 * g + j
                            jsl = slice(j * 512, (j + 1) * 512)
                            nc.tensor.matmul(
                                oacc[hl][:], vexts[hl][:, kt, :], pt[:, jsl],
                                start=(kt == 0), stop=False,
                            )
                            nc.tensor.matmul(
                                oacc[hl][:], vnegs[hl][:, kt, :],
                                mts[(j, hl)][:],
                                start=False, stop=(kt == KT - 1),
                            )
                # normalize: rows 0:64 are sum(p*v), row 64 is sum(p)
                for hl in range(HPC):
                    rec = sb.tile([1, 512], F32, tag=f"rec{hl}", bufs=2,
                                  name="rec")
                    nc.vector.reciprocal(rec[:], oacc[hl][64:65, :])
                    bc = psum.tile([64, 512], F32, tag=f"ps{hl}", bufs=2,
                                   name="bc")
                    nc.tensor.matmul(bc[:], ones_col64[:], rec[:],
                                     start=True, stop=True)
                    bcs = sb.tile([64, 512], F32, tag=f"bcs{hl}", bufs=2,
                                  name="bcs")
                    nc.scalar.copy(bcs[:], bc[:])
                    nc.vector.tensor_tensor(
                        outTn_sb[hl * 64 : (hl + 1) * 64, qsl],
                        oacc[hl][0:64, :], bcs[:], OP.mult,
                    )
            # scatter token chunks to the AllToAll input layout
            for g in range(NC):
                nc.sync.dma_start(
                    a2a_in[g, b], outTn_sb[:, g * TPC : (g + 1) * TPC]
                )

    # =======================================================================
    # Phase 2: AllToAll + token-parallel epilogue
    # =======================================================================
    with tile.TileContext(nc) as tc, contextlib.ExitStack() as ctx:
        consts = ctx.enter_context(tc.tile_pool(name="consts2", bufs=1))
        sb = ctx.enter_context(tc.tile_pool(name="sb2", bufs=1))
        psum = ctx.enter_context(tc.tile_pool(name="psum2", bufs=1, space="PSUM"))

        nc.gpsimd.collective_compute(
            "AllToAll",
            OP.bypass,
            replica_groups=[list(range(NC))],
            ins=[a2a_in[:]],
            outs=[a2a_out[:]],
        )

        ones_row = consts.tile([1, 128], F32)
        nc.vector.memset(ones_row[:], 1.0)
        bo_sb = consts.tile([1, D], F32)
        nc.sync.dma_start(bo_sb[:], bo[:])
        gamma_sb = consts.tile([1, D], F32)
        nc.sync.dma_start(gamma_sb[:], gamma[:])
        beta_sb = consts.tile([1, D], F32)
        nc.sync.dma_start(beta_sb[:], beta[:])

        gammab = consts.tile([128, D], F32)
        betab = consts.tile([128, D], F32)
        for dc in range(2):
            dsl = slice(dc * 512, (dc + 1) * 512)
            gps = psum.tile([128, 512], F32, tag="pp", bufs=2, name="gps")
            nc.tensor.matmul(gps[:], ones_row[:], gamma_sb[:, dsl],
                             start=True, stop=True)
            nc.scalar.copy(gammab[:, dsl], gps[:])
            bps = psum.tile([128, 512], F32, tag="pp", bufs=2, name="bps")
            nc.tensor.matmul(bps[:], ones_row[:], beta_sb[:, dsl],
                             start=True, stop=True)
            nc.scalar.copy(betab[:, dsl], bps[:])

        for b in range(B):
            for tt in range(TPC // 128):
                tsl = slice(tt * 128, (tt + 1) * 128)
                # the 8 head-group tiles [hd=128, tok=128] for this slice
                g8 = sb.tile([128, NC, 128], F32, tag="g8", bufs=2, name="g8")
                nc.sync.dma_start(
                    g8[:],
                    a2a_out[:, b, :, tsl].rearrange("g p t -> p g t"),
                )
                qres_sb = sb.tile([128, D], F32, tag="qres", bufs=2,
                                  name="qres_sb")
                nc.sync.dma_start(qres_sb[:], qres[b, tsl, :])

                xo = sb.tile([128, D], F32, tag="xo", bufs=2, name="xo")
                for dc in range(2):
                    dsl = slice(dc * 512, (dc + 1) * 512)
                    wo8 = sb.tile([128, NC, 512], F32, tag="wo8", bufs=2,
                                  name="wo8")
                    nc.sync.dma_start(
                        wo8[:],
                        wot.rearrange("(o p) n -> p o n", p=128)[:, :, dsl],
                    )
                    po = psum.tile([128, 512], F32, tag="pp", bufs=2,
                                   name="po")
                    for g in range(NC):
                        nc.tensor.matmul(
                            po[:], g8[:, g, :], wo8[:, g, :],
                            start=(g == 0), stop=False,
                        )
                    nc.tensor.matmul(
                        po[:], ones_row[:], bo_sb[:, dsl],
                        start=False, stop=True,
                    )
                    # residual add fused with psum evacuation
                    nc.vector.tensor_tensor(
                        xo[:, dsl], po[:], qres_sb[:, dsl], OP.add
                    )

                # ---- LayerNorm over the free (dmodel) axis --------------
                sumr = sb.tile([128, 1], F32, tag="sumr", bufs=2, name="sumr")
                nc.vector.tensor_reduce(
                    sumr[:], xo[:], mybir.AxisListType.X, OP.add
                )
                negmean = sb.tile([128, 1], F32, tag="negmean", bufs=2,
                                  name="negmean")
                nc.vector.tensor_scalar_mul(negmean[:], sumr[:], -1.0 / D)
                y = sb.tile([128, D], F32, tag="y", bufs=2, name="y")
                nc.vector.tensor_scalar_add(y[:], xo[:], negmean[:])
                sq = sb.tile([128, D], F32, tag="sq", bufs=2, name="sq")
                vsum = sb.tile([128, 1], F32, tag="vsum", bufs=2, name="vsum")
                nc.scalar.activation(sq[:], y[:], AF.Square,
                                     accum_out=vsum[:])
                v2 = sb.tile([128, 1], F32, tag="v2", bufs=2, name="v2")
                nc.vector.tensor_scalar(
                    v2[:], vsum[:], 1.0 / D, LN_EPS, OP.mult, OP.add
                )
                lnv = sb.tile([128, 1], F32, tag="lnv", bufs=2, name="lnv")
                nc.scalar.activation(lnv[:], v2[:], AF.Ln)
                rstd = sb.tile([128, 1], F32, tag="rstd", bufs=2, name="rstd")
                nc.scalar.activation(rstd[:], lnv[:], AF.Exp, scale=-0.5)
                # out = (y * rstd) * gammab + betab
                yg = sb.tile([128, D], F32, tag="yg", bufs=2, name="yg")
                nc.vector.scalar_tensor_tensor(
                    yg[:], y[:], rstd[:], gammab[:], OP.mult, OP.mult
                )
                fin = sb.tile([128, D], F32, tag="fin", bufs=2, name="fin")
                nc.vector.tensor_tensor(fin[:], yg[:], betab[:], OP.add)
                nc.sync.dma_start(out_c[b, tsl, :], fin[:])

    _fixup_sync_waits(nc)
    return nc


_CACHED_NC = None


def _get_nc():
    global _CACHED_NC
    if _CACHED_NC is None:
        _CACHED_NC = _build_bass()
    return _CACHED_NC


def _prepare_in_maps(queries, keys, values, attention_mask, attention_weights,
                     Wq, bq, Wk, bk, Wv, bv, Wo, bo, gamma, beta):
    queries = np.asarray(queries, np.float32)
    keys = np.asarray(keys, np.float32)
    values = np.asarray(values, np.float32)
    attention_mask = np.asarray(attention_mask)
    attention_weights = np.asarray(attention_weights, np.float32)
    Wq = np.asarray(Wq, np.float32)
    Wk = np.asarray(Wk, np.float32)
    Wv = np.asarray(Wv, np.float32)
    Wo = np.asarray(Wo, np.float32)
    bq = np.asarray(bq, np.float32)
    bk = np.asarray(bk, np.float32)
    bv = np.asarray(bv, np.float32)
    bo = np.asarray(bo, np.float32)
    gamma = np.asarray(gamma, np.float32)
    beta = np.asarray(beta, np.float32)

    xtq = np.ascontiguousarray(queries.transpose(0, 2, 1))
    xtk = np.ascontiguousarray(keys.transpose(0, 2, 1))
    xtv = np.ascontiguousarray(values.transpose(0, 2, 1))

    # wz = w * (1-mask) / 8 with exact zeros ONLY at masked positions.
    # (guard against accidental exact-zero weights at unmasked positions,
    # which would be misread as masked by the on-device wz==0 test)
    scale = np.float32(1.0 / np.sqrt(DH))
    wz_all = np.where(
        attention_mask, np.float32(0.0),
        np.maximum(attention_weights, np.float32(1e-30)) * scale,
    ).astype(np.float32)

    wot_full = np.ascontiguousarray(Wo.T)

    in_maps = []
    for c in range(NC):
        h0 = HPC * c
        # [B, HPC, NK, NQ] transposed blocks
        wzt = np.ascontiguousarray(
            wz_all[:, h0 : h0 + HPC].transpose(0, 1, 3, 2)
        )
        sl = slice(128 * c, 128 * (c + 1))
        in_maps.append({
            "xtq": xtq, "xtk": xtk, "xtv": xtv,
            "wzt": wzt,
            "wqt": np.ascontiguousarray(Wq[sl, :].T),
            "wkt": np.ascontiguousarray(Wk[sl, :].T),
            "wvt": np.ascontiguousarray(Wv[sl, :].T),
            "wot": wot_full,
            "bq": np.ascontiguousarray(bq[sl].reshape(128, 1)),
            "bk": np.ascontiguousarray(bk[sl].reshape(128, 1)),
            "bv": np.ascontiguousarray(bv[sl].reshape(1, 128)),
            "bo": np.ascontiguousarray(bo.reshape(1, D)),
            "gamma": np.ascontiguousarray(gamma.reshape(1, D)),
            "beta": np.ascontiguousarray(beta.reshape(1, D)),
            "qres": np.ascontiguousarray(
                np.stack([queries[bb, TPC * c : TPC * (c + 1), :]
                          for bb in range(B)])
            ),
        })
    return in_maps


class _Runner:
    """One-time jit of the SPMD bass program; callable many times.

    Mirrors bass2jax.run_bass_via_pjrt but hoists the jitted executable and
    (optionally) device-resident inputs so repeated calls don't re-lower or
    re-upload.
    """

    def __init__(self, nc):
        import jax
        from jax.sharding import Mesh, PartitionSpec
        from jax.experimental.shard_map import shard_map
        from concourse import bass2jax
        from concourse import mybir as _mybir

        bass2jax.install_neuronx_cc_hook()
        self.jax = jax
        self.nc = nc
        partition_name = (
            nc.partition_id_tensor.name if nc.partition_id_tensor else None
        )
        in_names, out_names, out_avals, zero_outs = [], [], [], []
        for alloc in nc.m.functions[0].allocations:
            if not isinstance(alloc, _mybir.MemoryLocationSet):
                continue
            name = alloc.memorylocations[0].name
            if alloc.kind == "ExternalInput":
                if name != partition_name:
                    in_names.append(name)
            elif alloc.kind == "ExternalOutput":
                shape = tuple(alloc.tensor_shape)
                dtype = _mybir.dt.np(alloc.dtype)
                out_names.append(name)
                out_avals.append(jax.core.ShapedArray(shape, dtype))
                zero_outs.append(np.zeros(shape, dtype))
        self.n_params = len(in_names)
        self.out_names = out_names
        self.out_avals = out_avals
        self.zero_outs = zero_outs
        all_in_names = list(in_names) + list(out_names)
        if partition_name is not None:
            all_in_names.append(partition_name)
        self.in_names = in_names

        def _body(*args):
            operands = list(args)
            if partition_name is not None:
                operands.append(bass2jax.partition_id_tensor())
            outs = bass2jax._bass_exec_p.bind(
                *operands,
                out_avals=tuple(out_avals),
                in_names=tuple(all_in_names),
                out_names=tuple(out_names),
                lowering_input_output_aliases=(),
                sim_require_finite=True,
                sim_require_nnan=True,
                nc=nc,
            )
            return tuple(outs)

        devices = jax.devices()[:NC]
        self.mesh = Mesh(np.asarray(devices), ("core",))
        n_outs = len(out_names)
        in_specs = (PartitionSpec("core"),) * (self.n_params + n_outs)
        out_specs = (PartitionSpec("core"),) * n_outs
        self.sharded = jax.jit(
            shard_map(_body, mesh=self.mesh, in_specs=in_specs,
                      out_specs=out_specs, check_rep=False),
            keep_unused=True,
        )
        self._dev_args = None

    def put_inputs(self, in_maps):
        """Upload per-core inputs (+ zero output buffers) to the devices."""
        concat_in = [
            np.concatenate([np.asarray(in_maps[c][n]) for c in range(NC)], axis=0)
            for n in self.in_names
        ]
        concat_zero = [
            np.zeros((NC * z.shape[0], *z.shape[1:]), z.dtype)
            for z in self.zero_outs
        ]
        self._dev_args = [self.jax.device_put(a) for a in concat_in + concat_zero]
        for a in self._dev_args:
            a.block_until_ready()

    def execute(self):
        outs = self.sharded(*self._dev_args)
        for o in outs:
            o.block_until_ready()
        return outs

    def results(self, outs):
        res = []
        for c in range(NC):
            res.append({
                name: np.asarray(outs[i]).reshape(NC, *self.out_avals[i].shape)[c]
                for i, name in enumerate(self.out_names)
            })
        return res


_CACHED_RUNNER = None


def _get_runner():
    global _CACHED_RUNNER
    if _CACHED_RUNNER is None:
        _CACHED_RUNNER = _Runner(_get_nc())
    return _CACHED_RUNNER


def kernel(**inputs) -> np.ndarray:
    runner = _get_runner()
    in_maps = _prepare_in_maps(**inputs)
    runner.put_inputs(in_maps)
    res = runner.results(runner.execute())
    out = np.empty((B, NQ, D), np.float32)
    for c in range(NC):
        oc = res[c]["out_c"]
        for b in range(B):
            out[b, TPC * c : TPC * (c + 1), :] = oc[b]
    return out


# revision 22
# speedup vs baseline: 28.1650x; 28.1650x over previous
"""Trainium2 Bass kernel for nn_MultiHeadAttention (B=2, Nq=Nk=2048, D=1024, H=16).

Sharding: head-parallel across 8 cores (2 heads/core, both batches) for the
projections + attention, then an AllToAll redistributes the tiny per-head
context vectors so the epilogue (Wo + bias + residual + LayerNorm) runs
token-parallel (256 tokens/batch/core).

Key design points:
 - scores are computed transposed (s^T[k,q] = kT.T @ qT, contraction d=64
   with the core's two heads row-packed into disjoint PE row groups), so the
   huge attention_weights tensor is loaded in a host-pre-transposed layout
   and P^T feeds the P@V matmul directly as the PE moving operand.
 - softmax denominators come from an extra ones-column in the stationary
   [V | 1] operand (no separate reduction pass).
 - masking: the host builds wz = w*(1-mask)/8 with exact zeros where masked;
   exp then yields exactly 1 there, and an accumulated matmul of [-V | -1]
   against the 0/1 mask (recomputed on-device as wz==0 by the otherwise-idle
   GPSIMD) removes those contributions exactly. No max-subtraction is needed
   (|q.k * w / 8| is O(6), far from exp overflow).
 - two sequential TileContexts (attention / epilogue) so SBUF pools are
   reused across the phases.
"""
import sys

sys.path.insert(0, "/opt/trn_rl_repo")

import contextlib

import numpy as np

import concourse.bass as bass
import concourse.mybir as mybir
import concourse.tile as tile
from concourse.vector_clock import ScopedClock

# ---------------------------------------------------------------------------
# Workaround: this walrus build accepts only ONE sync-wait on CTRL (Drain)
# instructions; the Tile tail drain can carry several. Split the surplus
# waits across trailing sync NOPs (they still run before the sem reset).
# ---------------------------------------------------------------------------
_MAX_WAITS = 1


def _split_waits(inst, nc):
    si = inst.sync_info
    if si is None or not si.on_wait or len(si.on_wait) <= _MAX_WAITS:
        return
    waits = list(si.on_wait)
    inst.sync_info = mybir.SyncInfo(
        on_wait=waits[:_MAX_WAITS], on_update=list(si.on_update)
    )
    for i in range(_MAX_WAITS, len(waits), _MAX_WAITS):
        nop = nc.sync.nop(nofuse=True)
        nsi = nop.ins.sync_info
        upd = list(nsi.on_update) if nsi is not None else []
        nop.ins.sync_info = mybir.SyncInfo(
            on_wait=waits[i : i + _MAX_WAITS], on_update=upd
        )


def _patched_drain_and_barrier(self, tick_clock, wait_clock):
    nc = self.nc
    drain_inst = nc.sync.drain()
    wait_clock.add_sem_waits(
        drain_inst.ins, ScopedClock({None: tick_clock.global_clock})
    )
    _split_waits(drain_inst.ins, nc)
    nc.all_engine_barrier()
    assert self.sems is not None
    popped = nc._tile_sem_poison_stack.pop()
    assert popped is self._sem_poison
    nc.clear_and_free_semaphores(list(self.sems.allocated().values()))
    nc.all_engine_barrier()


tile.TileContext._drain_and_barrier = _patched_drain_and_barrier


def _fixup_sync_waits(nc, max_waits=1):
    """Split any instruction carrying more than `max_waits` semaphore waits
    into same-engine NOPs placed immediately before it (this walrus build
    rejects multi-wait instructions)."""
    n_split = 0
    for f in nc.m.functions:
        for blk in f.blocks:
            insts = blk.instructions
            new = []
            for ins in insts:
                si = getattr(ins, "sync_info", None)
                if si is not None and si.on_wait and len(si.on_wait) > max_waits:
                    waits = list(si.on_wait)
                    head = waits[:-max_waits]
                    for k in range(0, len(head), max_waits):
                        nop = mybir.InstNoOp(
                            name=f"{ins.name}.wsplit{k}",
                            engine=ins.engine,
                            bass_nofuse=True,
                            sync_info=mybir.SyncInfo(
                                on_wait=head[k : k + max_waits], on_update=[]
                            ),
                        )
                        new.append(nop)
                        n_split += 1
                    ins.sync_info = mybir.SyncInfo(
                        on_wait=waits[-max_waits:], on_update=list(si.on_update)
                    )
                new.append(ins)
            if len(new) != len(insts):
                insts[:] = new
    return n_split

# ---------------------------------------------------------------------------

F32 = mybir.dt.float32
AF = mybir.ActivationFunctionType
OP = mybir.AluOpType

B, H, NQ, NK, D, DH = 2, 16, 2048, 2048, 1024, 64
NC = 8
HPC = H // NC          # heads per core = 2
TPC = NQ // NC         # epilogue tokens per core per batch = 256
LN_EPS = 1e-5

KT = NK // 128         # 16 k-tiles
QC = NQ // 512         # 4 q-chunks
DT = D // 128          # 8 dmodel tiles


def _build_bass(repeat1=1, repeat2=1, corr_mode="dve", ablate=()):
    """corr_mode: how the mask correction is applied.
      "gpsimd": ptm = mask - p on GPSIMD, single PV matmul chain.
      "pe":     extra [-V|-1] x mask matmuls accumulated on the PE.
      "none":   no mask handling (WRONG results; ablation timing only).
    ablate: subset of {"wzdma","dvemul","iseq","exp","pv"} -- removes that
      component (results wrong; for timing attribution only).
    """
    ablate = frozenset(ablate)
    nc = bass.Bass(num_devices=NC)

    # ---- parameters -------------------------------------------------------
    xtq = nc.declare_dram_parameter("xtq", [B, D, NQ], F32, isOutput=False)
    xtk = nc.declare_dram_parameter("xtk", [B, D, NK], F32, isOutput=False)
    xtv = nc.declare_dram_parameter("xtv", [B, D, NK], F32, isOutput=False)
    wzt = nc.declare_dram_parameter("wzt", [B, HPC, NK, NQ], F32, isOutput=False)
    wqt = nc.declare_dram_parameter("wqt", [D, 128], F32, isOutput=False)
    wkt = nc.declare_dram_parameter("wkt", [D, 128], F32, isOutput=False)
    wvt = nc.declare_dram_parameter("wvt", [D, 128], F32, isOutput=False)
    wot = nc.declare_dram_parameter("wot", [D, D], F32, isOutput=False)
    bq = nc.declare_dram_parameter("bq", [128, 1], F32, isOutput=False)
    bk = nc.declare_dram_parameter("bk", [128, 1], F32, isOutput=False)
    bv = nc.declare_dram_parameter("bv", [1, 128], F32, isOutput=False)
    bo = nc.declare_dram_parameter("bo", [1, D], F32, isOutput=False)
    gamma = nc.declare_dram_parameter("gamma", [1, D], F32, isOutput=False)
    beta = nc.declare_dram_parameter("beta", [1, D], F32, isOutput=False)
    qres = nc.declare_dram_parameter("qres", [B, TPC, D], F32, isOutput=False)
    out_c = nc.declare_dram_parameter("out_c", [B, TPC, D], F32, isOutput=True)

    # DRAM scratch shared across the two tile contexts.
    # a2a_in[g, b, :, :] = this core's heads, token chunk destined to core g.
    # After AllToAll: a2a_out[g, b, :, :] = core g's heads for THIS core's
    # token slice.
    a2a_in = nc.dram_tensor("a2a_in", [NC, B, 128, TPC], F32)
    a2a_out = nc.dram_tensor("a2a_out", [NC, B, 128, TPC], F32)

    # =======================================================================
    # Phase 1: projections + attention (head-parallel)
    # =======================================================================
    with tile.TileContext(nc) as tc, contextlib.ExitStack() as ctx:
        consts = ctx.enter_context(tc.tile_pool(name="consts", bufs=1))
        sb = ctx.enter_context(tc.tile_pool(name="sb", bufs=1))
        psum = ctx.enter_context(tc.tile_pool(name="psum", bufs=1, space="PSUM"))

        ones_row = consts.tile([1, 128], F32)
        nc.vector.memset(ones_row[:], 1.0)
        ones_col64 = consts.tile([1, 64], F32)
        nc.vector.memset(ones_col64[:], 1.0)

        wq_sb = consts.tile([128, DT, 128], F32)
        nc.sync.dma_start(wq_sb[:], wqt.rearrange("(o p) m -> p o m", p=128))
        wk_sb = consts.tile([128, DT, 128], F32)
        nc.sync.dma_start(wk_sb[:], wkt.rearrange("(o p) m -> p o m", p=128))
        wv_sb = consts.tile([128, DT, 128], F32)
        nc.sync.dma_start(wv_sb[:], wvt.rearrange("(o p) m -> p o m", p=128))

        bq_sb = consts.tile([128, 1], F32)
        nc.sync.dma_start(bq_sb[:], bq[:])
        bk_sb = consts.tile([128, 1], F32)
        nc.sync.dma_start(bk_sb[:], bk[:])
        bv_sb = consts.tile([1, 128], F32)
        nc.sync.dma_start(bv_sb[:], bv[:])

        for b in [bb for _ in range(repeat1) for bb in range(B)]:
            # ---- Q/K projections, transposed layout [hd, tok] ------------
            qt_sb = sb.tile([128, NQ], F32, tag="qt_sb", name="qt_sb")
            kt_sb = sb.tile([128, NK], F32, tag="kt_sb", name="kt_sb")
            for src, dst, w_sb, b_ap in (
                (xtq, qt_sb, wq_sb, bq_sb),
                (xtk, kt_sb, wk_sb, bk_sb),
            ):
                for half in range(2):
                    hslc = slice(half * 1024, (half + 1) * 1024)
                    pj = [
                        psum.tile([128, 512], F32, tag="pp", bufs=2, name="pj")
                        for _ in range(2)
                    ]
                    for kd in range(DT):
                        xs = sb.tile([128, 1024], F32, tag="xs", bufs=3,
                                     name="xs")
                        nc.sync.dma_start(
                            xs[:], src[b, kd * 128 : (kd + 1) * 128, hslc]
                        )
                        for i in range(2):
                            nc.tensor.matmul(
                                pj[i][:], w_sb[:, kd, :],
                                xs[:, i * 512 : (i + 1) * 512],
                                start=(kd == 0), stop=(kd == DT - 1),
                            )
                    for i in range(2):
                        tsl = slice(half * 1024 + i * 512,
                                    half * 1024 + (i + 1) * 512)
                        nc.scalar.activation(
                            dst[:, tsl], pj[i][:], AF.Identity,
                            bias=b_ap[:], scale=1.0,
                        )

            # ---- V projection, natural layout [tok, hd]; build [-V|-1] per
            # head. The PV matmul streams (mask - p), so the negated
            # stationary yields sum((p-mask)*v) and, via the -1 column,
            # sum(p-mask) -- the softmax denominator.
            vnegs = []
            vexts = []
            for hl in range(HPC):
                vn = sb.tile([128, KT, 65], F32, tag=f"vneg{hl}", name="vn")
                nc.vector.memset(vn[:, :, 64:65], -1.0)
                vnegs.append(vn)
                if corr_mode == "pe":
                    ve = sb.tile([128, KT, 65], F32, tag=f"vext{hl}", name="ve")
                    nc.vector.memset(ve[:, :, 64:65], 1.0)
                    vexts.append(ve)
            for tt in range(KT):
                xv8 = sb.tile([128, DT, 128], F32, tag="xv8", bufs=2,
                              name="xv8")
                nc.sync.dma_start(
                    xv8[:],
                    xtv[b].rearrange("(o p) t -> p o t", p=128)[
                        :, :, tt * 128 : (tt + 1) * 128
                    ],
                )
                pv = psum.tile([128, 128], F32, tag="pp", bufs=2, name="pv")
                for kd in range(DT):
                    nc.tensor.matmul(
                        pv[:], xv8[:, kd, :], wv_sb[:, kd, :],
                        start=(kd == 0), stop=False,
                    )
                nc.tensor.matmul(
                    pv[:], ones_row[:, 0:128], bv_sb[:],
                    start=False, stop=True,
                )
                for hl in range(HPC):
                    nc.vector.tensor_scalar_mul(
                        vnegs[hl][:, tt, 0:64],
                        pv[:, hl * 64 : hl * 64 + 64], -1.0,
                    )
                    if corr_mode == "pe":
                        nc.scalar.copy(
                            vexts[hl][:, tt, 0:64],
                            pv[:, hl * 64 : hl * 64 + 64],
                        )

            # ---- attention ----------------------------------------------
            outTn_sb = sb.tile([128, NQ], F32, tag="outTn", name="outTn_sb")
            if "pv" in ablate:
                nc.vector.memset(outTn_sb[:], 0.0)
            wz_c = None
            if "wzdma" in ablate:
                wz_c = sb.tile([128, 512], F32, tag="wz_c", name="wz_c")
                nc.sync.dma_start(wz_c[:], wzt[0, 0, 0:128, 0:512])
            for qc in range(QC):
                qsl = slice(qc * 512, (qc + 1) * 512)
                oacc = [
                    psum.tile([65, 512], F32, tag=f"o{hl}", name="oacc")
                    for hl in range(HPC)
                ]
                for g in range(KT // 2):
                    texp = [
                        sb.tile([128, 1024], F32, tag=f"t{hl}", bufs=2,
                                name="texp")
                        for hl in range(HPC)
                    ]
                    mtg = [
                        sb.tile([128, 1024], F32, tag=f"mt{hl}", bufs=2,
                                name="mtg")
                        for hl in range(HPC)
                    ]
                    wzs = {}
                    for j in range(2):
                        kt = 2 * g + j
                        ksl = slice(kt * 128, (kt + 1) * 128)
                        jsl = slice(j * 512, (j + 1) * 512)
                        for hl in range(HPC):
                            hsl = slice(hl * 64, (hl + 1) * 64)
                            if "wzdma" in ablate:
                                wz = wz_c
                            else:
                                wz = sb.tile([128, 512], F32, tag=f"wz{hl}",
                                             bufs=4, name="wz")
                                # split DMA issue across the two HWDGE rings
                                dma_eng = nc.sync if hl == 0 else nc.scalar
                                dma_eng.dma_start(wz[:], wzt[b, hl, ksl, qsl])
                            wzs[(j, hl)] = wz
                            if corr_mode != "none" and "iseq" not in ablate:
                                # mask = (wz == 0); DVE tensor_scalar (2x)
                                nc.vector.tensor_scalar(
                                    mtg[hl][:, jsl], wz[:], 0.0, None,
                                    OP.is_equal,
                                )
                            ps = psum.tile([128, 512], F32, tag=f"ps{hl}",
                                           bufs=2, name="ps")
                            nc.tensor.matmul(
                                ps[:], kt_sb[hsl, ksl], qt_sb[hsl, qsl],
                                start=True, stop=True,
                                tile_position=(hl * 64, 0),
                            )
                            if "dvemul" not in ablate:
                                nc.vector.tensor_tensor(
                                    texp[hl][:, jsl], ps[:], wz[:], OP.mult
                                )
                    for hl in range(HPC):
                        pt = sb.tile([128, 1024], F32, tag=f"pt{hl}", bufs=2,
                                     name="pt")
                        if "exp" in ablate:
                            pt = texp[hl]
                        elif "dvemul" in ablate:
                            for j in range(2):
                                jsl = slice(j * 512, (j + 1) * 512)
                                nc.scalar.activation(
                                    pt[:, jsl], wzs[(j, hl)][:], AF.Exp
                                )
                        else:
                            nc.scalar.activation(pt[:], texp[hl][:], AF.Exp)
                        if corr_mode in ("gpsimd", "dve"):
                            # ptm = mask - p
                            ptm = sb.tile([128, 1024], F32, tag=f"pm{hl}",
                                          bufs=2, name="ptm")
                            eng = nc.gpsimd if corr_mode == "gpsimd" else nc.vector
                            eng.tensor_tensor(
                                ptm[:], mtg[hl][:], pt[:], OP.subtract
                            )
                        if "pv" in ablate:
                            continue
                        for j in range(2):
                            kt = 2 * g + j
                            jsl = slice(j * 512, (j + 1) * 512)
                            if corr_mode in ("gpsimd", "dve"):
                                nc.tensor.matmul(
                                    oacc[hl][:], vnegs[hl][:, kt, :],
                                    ptm[:, jsl],
                                    start=(kt == 0), stop=(kt == KT - 1),
                                )
                            elif corr_mode == "pe":
                                nc.tensor.matmul(
                                    oacc[hl][:], vexts[hl][:, kt, :],
                                    pt[:, jsl],
                                    start=(kt == 0), stop=False,
                                )
                                nc.tensor.matmul(
                                    oacc[hl][:], vnegs[hl][:, kt, :],
                                    mtg[hl][:, jsl],
                                    start=False, stop=(kt == KT - 1),
                                )
                            else:  # none (ablation)
                                nc.tensor.matmul(
                                    oacc[hl][:], vnegs[hl][:, kt, :],
                                    pt[:, jsl],
                                    start=(kt == 0), stop=(kt == KT - 1),
                                )
                # normalize: rows 0:64 are sum(p*v), row 64 is sum(p)
                if "pv" in ablate:
                    continue
                for hl in range(HPC):
                    rec = sb.tile([1, 512], F32, tag=f"rec{hl}", bufs=2,
                                  name="rec")
                    nc.vector.reciprocal(rec[:], oacc[hl][64:65, :])
                    bc = psum.tile([64, 512], F32, tag=f"ps{hl}", bufs=2,
                                   name="bc")
                    nc.tensor.matmul(bc[:], ones_col64[:], rec[:],
                                     start=True, stop=True)
                    bcs = sb.tile([64, 512], F32, tag=f"bcs{hl}", bufs=2,
                                  name="bcs")
                    nc.scalar.copy(bcs[:], bc[:])
                    nc.vector.tensor_tensor(
                        outTn_sb[hl * 64 : (hl + 1) * 64, qsl],
                        oacc[hl][0:64, :], bcs[:], OP.mult,
                    )
            # scatter token chunks to the AllToAll input layout
            for g in range(NC):
                nc.sync.dma_start(
                    a2a_in[g, b], outTn_sb[:, g * TPC : (g + 1) * TPC]
                )

    # =======================================================================
    # Phase 2: AllToAll + token-parallel epilogue
    # =======================================================================
    with tile.TileContext(nc) as tc, contextlib.ExitStack() as ctx:
        consts = ctx.enter_context(tc.tile_pool(name="consts2", bufs=1))
        sb = ctx.enter_context(tc.tile_pool(name="sb2", bufs=1))
        psum = ctx.enter_context(tc.tile_pool(name="psum2", bufs=1, space="PSUM"))

        nc.gpsimd.collective_compute(
            "AllToAll",
            OP.bypass,
            replica_groups=[list(range(NC))],
            ins=[a2a_in[:]],
            outs=[a2a_out[:]],
        )

        ones_row = consts.tile([1, 128], F32)
        nc.vector.memset(ones_row[:], 1.0)
        bo_sb = consts.tile([1, D], F32)
        nc.sync.dma_start(bo_sb[:], bo[:])
        gamma_sb = consts.tile([1, D], F32)
        nc.sync.dma_start(gamma_sb[:], gamma[:])
        beta_sb = consts.tile([1, D], F32)
        nc.sync.dma_start(beta_sb[:], beta[:])

        gammab = consts.tile([128, D], F32)
        betab = consts.tile([128, D], F32)
        for dc in range(2):
            dsl = slice(dc * 512, (dc + 1) * 512)
            gps = psum.tile([128, 512], F32, tag="pp", bufs=2, name="gps")
            nc.tensor.matmul(gps[:], ones_row[:], gamma_sb[:, dsl],
                             start=True, stop=True)
            nc.scalar.copy(gammab[:, dsl], gps[:])
            bps = psum.tile([128, 512], F32, tag="pp", bufs=2, name="bps")
            nc.tensor.matmul(bps[:], ones_row[:], beta_sb[:, dsl],
                             start=True, stop=True)
            nc.scalar.copy(betab[:, dsl], bps[:])

        # Wo resident for the whole epilogue (read once)
        wo_sb = consts.tile([128, NC, D], F32)
        nc.sync.dma_start(wo_sb[:], wot.rearrange("(o p) n -> p o n", p=128))

        for b in [bb for _ in range(repeat2) for bb in range(B)]:
            for tt in range(TPC // 128):
                tsl = slice(tt * 128, (tt + 1) * 128)
                # the 8 head-group tiles [hd=128, tok=128] for this slice
                g8 = sb.tile([128, NC, 128], F32, tag="g8", bufs=2, name="g8")
                nc.sync.dma_start(
                    g8[:],
                    a2a_out[:, b, :, tsl].rearrange("g p t -> p g t"),
                )
                qres_sb = sb.tile([128, D], F32, tag="qres", bufs=2,
                                  name="qres_sb")
                nc.sync.dma_start(qres_sb[:], qres[b, tsl, :])

                xo = sb.tile([128, D], F32, tag="xo", bufs=2, name="xo")
                for dc in range(2):
                    dsl = slice(dc * 512, (dc + 1) * 512)
                    po = psum.tile([128, 512], F32, tag="pp", bufs=2,
                                   name="po")
                    for g in range(NC):
                        nc.tensor.matmul(
                            po[:], g8[:, g, :], wo_sb[:, g, dsl],
                            start=(g == 0), stop=False,
                        )
                    nc.tensor.matmul(
                        po[:], ones_row[:], bo_sb[:, dsl],
                        start=False, stop=True,
                    )
                    # residual add fused with psum evacuation
                    nc.vector.tensor_tensor(
                        xo[:, dsl], po[:], qres_sb[:, dsl], OP.add
                    )

                # ---- LayerNorm over the free (dmodel) axis --------------
                sumr = sb.tile([128, 1], F32, tag="sumr", bufs=2, name="sumr")
                nc.vector.tensor_reduce(
                    sumr[:], xo[:], mybir.AxisListType.X, OP.add
                )
                negmean = sb.tile([128, 1], F32, tag="negmean", bufs=2,
                                  name="negmean")
                nc.vector.tensor_scalar_mul(negmean[:], sumr[:], -1.0 / D)
                y = sb.tile([128, D], F32, tag="y", bufs=2, name="y")
                nc.vector.tensor_scalar_add(y[:], xo[:], negmean[:])
                sq = sb.tile([128, D], F32, tag="sq", bufs=2, name="sq")
                vsum = sb.tile([128, 1], F32, tag="vsum", bufs=2, name="vsum")
                nc.scalar.activation(sq[:], y[:], AF.Square,
                                     accum_out=vsum[:])
                v2 = sb.tile([128, 1], F32, tag="v2", bufs=2, name="v2")
                nc.vector.tensor_scalar(
                    v2[:], vsum[:], 1.0 / D, LN_EPS, OP.mult, OP.add
                )
                lnv = sb.tile([128, 1], F32, tag="lnv", bufs=2, name="lnv")
                nc.scalar.activation(lnv[:], v2[:], AF.Ln)
                rstd = sb.tile([128, 1], F32, tag="rstd", bufs=2, name="rstd")
                nc.scalar.activation(rstd[:], lnv[:], AF.Exp, scale=-0.5)
                # out = (y * rstd) * gammab + betab
                yg = sb.tile([128, D], F32, tag="yg", bufs=2, name="yg")
                nc.vector.scalar_tensor_tensor(
                    yg[:], y[:], rstd[:], gammab[:], OP.mult, OP.mult
                )
                fin = sb.tile([128, D], F32, tag="fin", bufs=2, name="fin")
                nc.vector.tensor_tensor(fin[:], yg[:], betab[:], OP.add)
                nc.sync.dma_start(out_c[b, tsl, :], fin[:])

    _fixup_sync_waits(nc)
    return nc


_CACHED_NC = None


def _get_nc():
    global _CACHED_NC
    if _CACHED_NC is None:
        _CACHED_NC = _build_bass()
    return _CACHED_NC


def _prepare_in_maps(queries, keys, values, attention_mask, attention_weights,
                     Wq, bq, Wk, bk, Wv, bv, Wo, bo, gamma, beta):
    queries = np.asarray(queries, np.float32)
    keys = np.asarray(keys, np.float32)
    values = np.asarray(values, np.float32)
    attention_mask = np.asarray(attention_mask)
    attention_weights = np.asarray(attention_weights, np.float32)
    Wq = np.asarray(Wq, np.float32)
    Wk = np.asarray(Wk, np.float32)
    Wv = np.asarray(Wv, np.float32)
    Wo = np.asarray(Wo, np.float32)
    bq = np.asarray(bq, np.float32)
    bk = np.asarray(bk, np.float32)
    bv = np.asarray(bv, np.float32)
    bo = np.asarray(bo, np.float32)
    gamma = np.asarray(gamma, np.float32)
    beta = np.asarray(beta, np.float32)

    xtq = np.ascontiguousarray(queries.transpose(0, 2, 1))
    xtk = np.ascontiguousarray(keys.transpose(0, 2, 1))
    xtv = np.ascontiguousarray(values.transpose(0, 2, 1))

    # wz = w * (1-mask) / 8 with exact zeros ONLY at masked positions.
    # (guard against accidental exact-zero weights at unmasked positions,
    # which would be misread as masked by the on-device wz==0 test)
    scale = np.float32(1.0 / np.sqrt(DH))
    wz_all = np.where(
        attention_mask, np.float32(0.0),
        np.maximum(attention_weights, np.float32(1e-30)) * scale,
    ).astype(np.float32)

    wot_full = np.ascontiguousarray(Wo.T)

    in_maps = []
    for c in range(NC):
        h0 = HPC * c
        # [B, HPC, NK, NQ] transposed blocks
        wzt = np.ascontiguousarray(
            wz_all[:, h0 : h0 + HPC].transpose(0, 1, 3, 2)
        )
        sl = slice(128 * c, 128 * (c + 1))
        in_maps.append({
            "xtq": xtq, "xtk": xtk, "xtv": xtv,
            "wzt": wzt,
            "wqt": np.ascontiguousarray(Wq[sl, :].T),
            "wkt": np.ascontiguousarray(Wk[sl, :].T),
            "wvt": np.ascontiguousarray(Wv[sl, :].T),
            "wot": wot_full,
            "bq": np.ascontiguousarray(bq[sl].reshape(128, 1)),
            "bk": np.ascontiguousarray(bk[sl].reshape(128, 1)),
            "bv": np.ascontiguousarray(bv[sl].reshape(1, 128)),
            "bo": np.ascontiguousarray(bo.reshape(1, D)),
            "gamma": np.ascontiguousarray(gamma.reshape(1, D)),
            "beta": np.ascontiguousarray(beta.reshape(1, D)),
            "qres": np.ascontiguousarray(
                np.stack([queries[bb, TPC * c : TPC * (c + 1), :]
                          for bb in range(B)])
            ),
        })
    return in_maps


class _Runner:
    """One-time jit of the SPMD bass program; callable many times.

    Mirrors bass2jax.run_bass_via_pjrt but hoists the jitted executable and
    (optionally) device-resident inputs so repeated calls don't re-lower or
    re-upload.
    """

    def __init__(self, nc):
        import jax
        from jax.sharding import Mesh, PartitionSpec
        from jax.experimental.shard_map import shard_map
        from concourse import bass2jax
        from concourse import mybir as _mybir

        bass2jax.install_neuronx_cc_hook()
        self.jax = jax
        self.nc = nc
        partition_name = (
            nc.partition_id_tensor.name if nc.partition_id_tensor else None
        )
        in_names, out_names, out_avals, zero_outs = [], [], [], []
        for alloc in nc.m.functions[0].allocations:
            if not isinstance(alloc, _mybir.MemoryLocationSet):
                continue
            name = alloc.memorylocations[0].name
            if alloc.kind == "ExternalInput":
                if name != partition_name:
                    in_names.append(name)
            elif alloc.kind == "ExternalOutput":
                shape = tuple(alloc.tensor_shape)
                dtype = _mybir.dt.np(alloc.dtype)
                out_names.append(name)
                out_avals.append(jax.core.ShapedArray(shape, dtype))
                zero_outs.append(np.zeros(shape, dtype))
        self.n_params = len(in_names)
        self.out_names = out_names
        self.out_avals = out_avals
        self.zero_outs = zero_outs
        all_in_names = list(in_names) + list(out_names)
        if partition_name is not None:
            all_in_names.append(partition_name)
        self.in_names = in_names

        def _body(*args):
            operands = list(args)
            if partition_name is not None:
                operands.append(bass2jax.partition_id_tensor())
            outs = bass2jax._bass_exec_p.bind(
                *operands,
                out_avals=tuple(out_avals),
                in_names=tuple(all_in_names),
                out_names=tuple(out_names),
                lowering_input_output_aliases=(),
                sim_require_finite=True,
                sim_require_nnan=True,
                nc=nc,
            )
            return tuple(outs)

        devices = jax.devices()[:NC]
        self.mesh = Mesh(np.asarray(devices), ("core",))
        n_outs = len(out_names)
        in_specs = (PartitionSpec("core"),) * (self.n_params + n_outs)
        out_specs = (PartitionSpec("core"),) * n_outs
        self.sharded = jax.jit(
            shard_map(_body, mesh=self.mesh, in_specs=in_specs,
                      out_specs=out_specs, check_rep=False),
            keep_unused=True,
        )
        self._dev_args = None

    def put_inputs(self, in_maps):
        """Upload per-core inputs (+ zero output buffers) to the devices."""
        concat_in = [
            np.concatenate([np.asarray(in_maps[c][n]) for c in range(NC)], axis=0)
            for n in self.in_names
        ]
        concat_zero = [
            np.zeros((NC * z.shape[0], *z.shape[1:]), z.dtype)
            for z in self.zero_outs
        ]
        self._dev_args = [self.jax.device_put(a) for a in concat_in + concat_zero]
        for a in self._dev_args:
            a.block_until_ready()

    def execute(self):
        outs = self.sharded(*self._dev_args)
        for o in outs:
            o.block_until_ready()
        return outs

    def results(self, outs):
        res = []
        for c in range(NC):
            res.append({
                name: np.asarray(outs[i]).reshape(NC, *self.out_avals[i].shape)[c]
                for i, name in enumerate(self.out_names)
            })
        return res


_CACHED_RUNNER = None


def _get_runner():
    global _CACHED_RUNNER
    if _CACHED_RUNNER is None:
        _CACHED_RUNNER = _Runner(_get_nc())
    return _CACHED_RUNNER


def kernel(**inputs) -> np.ndarray:
    runner = _get_runner()
    in_maps = _prepare_in_maps(**inputs)
    runner.put_inputs(in_maps)
    res = runner.results(runner.execute())
    out = np.empty((B, NQ, D), np.float32)
    for c in range(NC):
        oc = res[c]["out_c"]
        for b in range(B):
            out[b, TPC * c : TPC * (c + 1), :] = oc[b]
    return out
